# revision 28
# baseline (speedup 1.0000x reference)
"""Trainium2 Bass kernel for nn_ComposedCliffordSteerableKernel.

Computation (see reference): for each of 16x16 (m, n) block pairs, a tiny
3D conv (8,8,7^3) x (8,8,7^3) -> (8,8,7^3) with SAME padding, then
elementwise * shell * factor.

Both conv operands depend on the pair, so each pair is an independent
[M=8, K=8, N] matmul per spatial tap -- far too small for the 128x128 PE
array on its own.  Two packings are implemented:

- "f32r"/"f32" (_build_nc): per m-block (8 output rows), one 128x128
  block-diagonal matmul per tap: contraction partitions (n,j) = 16 pairs
  x 8 input blades, output partitions (n,q), free dim = spatial output
  positions of one batch-blade p (N=392, w padded to 8 for FP32R's even
  innermost-run rule).  8 PSUM banks (one per p) accumulate all 343
  taps.  float32r gives single-pass fp32 (1 cycle/row at N>=256) at
  ~tf32 precision (measured 1.4e-4 rel).

- "*t16" (_build_nc_t16): the PE is packed as 16 independent 32x32
  tiles.  Tile (row 32g, col 32c) contracts pair-group g (4 pairs) and
  writes PSUM strip c; pairing c = (g + t) % 4 over tap-classes
  t = lin % 4 uses all 16 tiles and quadruples useful MAC rate vs the
  block-diagonal scheme.  Per output depth od, 4 PSUM banks (one per
  class, od-parity double-buffered) accumulate the taps; output strip s
  is then sum over t of bank_t[strip (s+t)%4] (partition-crossed DVE
  adds).  Zero-contribution (od,kd) pairs are skipped and oh is
  restricted to its valid window (~1.75x fewer MACs).
  dtypes: "fp16t16" 1-pass fp16 (~3e-4 rel); "bf16t16" 1-pass bf16
  (~2e-3); "bf16x3t16" hi/lo-split 3-pass bf16 (~4e-6, fp32-grade).

k1 is held transposed (columns -> partitions) and zero-padded to
(13,13,14) so every tap is just an AP window offset; weights are
DMA-scattered into block-diagonal SBUF tiles whose off-diagonal zeros
persist from a one-time fill.  Sharding: core c takes output row-blocks
2c and 2c+1; no inter-core communication.
"""

import sys

for _p in ("/opt/trn_rl_repo",):
    if _p not in sys.path:
        sys.path.insert(0, _p)

import numpy as np

NB = 8
KS = 7
S3 = KS * KS * KS          # 343
WPAD = KS + 1              # 8 (even innermost run for fp32r)
SP = KS * KS * WPAD        # 392 psum free size per batch-blade
DPAD, HPAD, WPAD2 = 13, 13, 14
PADVOL = DPAD * HPAD * WPAD2   # 2366 per batch-blade in k1T
N_CORES = 8
M_PER_CORE = 2             # m-blocks per core

MODE = "f32r"              # "f32r" | "f32" | "bf16t16" | "bf16x3t16"

_CACHE = {}

SPT = KS * WPAD * NB       # 448: T16 psum free per od: (p, oh, ow8)


def _build_nc(mode):
    import concourse.bass as bass
    import concourse.tile as tile
    from concourse import bacc, mybir

    f32 = mybir.dt.float32
    f32r = mybir.dt.float32r
    mult = mybir.AluOpType.mult

    nc = bacc.Bacc("TRN2", target_bir_lowering=False, debug=False)

    # k1 arrives host-padded: [16 rows, 128 cols, 13*13*14] with the 7^3
    # interior at [3:10,3:10,3:10] (f32r tiles cannot be memset, so the
    # zero padding comes in via the cast DMA)
    k1 = nc.dram_tensor(
        "k1pad", [M_PER_CORE * NB, 128, PADVOL], f32, kind="ExternalInput"
    )
    k2 = nc.dram_tensor("k2", [M_PER_CORE * NB, 128, S3], f32, kind="ExternalInput")
    shell = nc.dram_tensor(
        "shell", [M_PER_CORE * NB, 128, SP], f32, kind="ExternalInput"
    )
    factor = nc.dram_tensor("factor", [128, 1], f32, kind="ExternalInput")
    zeros = nc.dram_tensor(
        "zeros", [128, 128 * KS * KS], f32, kind="ExternalInput"
    )
    out = nc.dram_tensor("out", [M_PER_CORE * NB, 128, SP], f32, kind="ExternalOutput")

    mm_dt = f32r if mode == "f32r" else f32

    with tile.TileContext(nc) as tc:
        with (
            tc.tile_pool(name="persist", bufs=1) as persist,
            tc.tile_pool(name="io", bufs=2) as io,
            tc.tile_pool(name="ps", bufs=1, space="PSUM") as pspool,
        ):
            # k1 transposed + zero padded: [(n,j)=128, p=8, 13, 13, 14]
            # stored as float32r so fp32r matmuls accept it (DMA casts)
            k1t = persist.tile([128, NB, DPAD, HPAD, WPAD2], mm_dt, tag="k1t")

            # two weight chunk slots, each one kd-plane of 49 taps:
            # [(n,j)=128, (n,q)=128, tap=49] (taps contiguous so the k2
            # DMA has a stride-1 final dim); zeros off the diagonal persist
            # from a one-time cast-DMA fill from the zeros input
            wslots = []
            for i in range(2):
                w = persist.tile([128, 128, KS * KS], mm_dt, tag=f"w{i}", name=f"w{i}")
                nc.gpsimd.dma_start(
                    out=w.rearrange("c a t -> c (a t)"), in_=zeros[:, :]
                )
                wslots.append(w)

            fac = persist.tile([128, 1], f32, tag="fac")
            nc.sync.dma_start(out=fac[:, :], in_=factor[:, :])

            psum = [
                pspool.tile([128, SP], f32, tag=f"pp{p}", name=f"pp{p}")
                for p in range(NB)
            ]

            for m in range(M_PER_CORE):
                # load k1 block (host-padded, transposed into partitions);
                # one contiguous cast DMA per batch-blade p
                for p in range(NB):
                    nc.gpsimd.dma_start(
                        out=k1t[:, p, :, :, :],
                        in_=k1[m * NB + p, :, :],
                    )

                # shell for this m (host pre-padded w->8, so contiguous),
                # pre-scaled by factor
                sh = io.tile([128, NB, SP], f32, tag="shell")
                nc.sync.dma_start(
                    out=sh[:, :, :],
                    in_=shell[m * NB:(m + 1) * NB, :, :].rearrange("p c s -> c p s"),
                )
                shf = io.tile([128, NB, SP], f32, tag="shellf")
                nc.vector.tensor_scalar_mul(shf[:, :, :], sh[:, :, :], fac[:, 0:1])

                for kd in range(KS):
                    w = wslots[kd % 2]
                    # load this kd-plane's 16 diagonal blocks:
                    # w[n*8+j, n*8+q, t] = k2[m*8+q, n*8+j, kd*49+t]
                    for n in range(16):
                        nc.gpsimd.dma_start(
                            out=w[n * NB:(n + 1) * NB, n * NB:(n + 1) * NB, :],
                            in_=k2[
                                m * NB:(m + 1) * NB,
                                n * NB:(n + 1) * NB,
                                kd * KS * KS:(kd + 1) * KS * KS,
                            ].rearrange("q j t -> j q t"),
                        )
                    for kh in range(KS):
                        for kw in range(KS):
                            t = kh * KS + kw
                            lhsT = w[:, :, t]
                            first = kd == 0 and t == 0
                            last = kd == KS - 1 and t == KS * KS - 1
                            for p in range(NB):
                                rhs = k1t[
                                    :, p, kd:kd + KS, kh:kh + KS, kw:kw + WPAD
                                ]
                                nc.tensor.matmul(
                                    psum[p][:, :],
                                    lhsT,
                                    rhs,
                                    start=first,
                                    stop=last,
                                )

                # evacuate: out = psum * factor * shell  (shell already
                # carries factor), then store
                ost = io.tile([128, NB, SP], f32, tag="ost")
                for p in range(NB):
                    nc.vector.tensor_mul(
                        ost[:, p, :], psum[p][:, :], shf[:, p, :]
                    )
                nc.sync.dma_start(
                    out=out[m * NB:(m + 1) * NB, :, :].rearrange("p c s -> c p s"),
                    in_=ost[:, :, :],
                )
    nc.compile()
    return nc


def _build_nc_t16(mode):
    """16x 32x32 PE-tile variant (bf16).

    Per m-block, per output depth od (7), accumulate all 343 taps into 4
    PSUM banks (one per tap-class t = lin%4), double-buffered by od
    parity.  Tile (row 32g, col 32c) contracts n-group g (SBUF partitions
    32g..32g+31 of k1t) and writes PSUM partitions 32c; pairing c =
    (g+t)%4 uses all 16 tiles.  Output strip s is then sum over t of
    bank_t[strip (s+t)%4] (partition-crossed DVE adds).

    mode "bf16t16": single-pass bf16 (input rounding ~2.5e-3 rel).
    mode "bf16x3t16": 3-pass hi/lo split (Ah*Wh + Ah*Wl + Al*Wh, ~5e-6).
    """
    import concourse.tile as tile
    from concourse import bacc, mybir

    f32 = mybir.dt.float32
    bf16 = (mybir.dt.float16 if mode == "fp16t16" else mybir.dt.bfloat16)
    npass = 3 if mode == "bf16x3t16" else 1

    nc = bacc.Bacc("TRN2", target_bir_lowering=False, debug=False)

    names = ["h"] if npass == 1 else ["h", "l"]
    k1d = {
        s: nc.dram_tensor(f"k1{s}", [M_PER_CORE * NB, 128, S3], bf16,
                          kind="ExternalInput")
        for s in names
    }
    k2d = {
        s: nc.dram_tensor(f"k2{s}", [M_PER_CORE * NB, 128, S3], bf16,
                          kind="ExternalInput")
        for s in names
    }
    shell = nc.dram_tensor(
        "shell", [M_PER_CORE * NB, 128, SP], f32, kind="ExternalInput"
    )
    factor = nc.dram_tensor("factor", [128, 1], f32, kind="ExternalInput")
    out = nc.dram_tensor("out", [M_PER_CORE * NB, 128, SP], f32, kind="ExternalOutput")

    # (weight-piece, k1-piece) per pass: h*h + h*l + l*h
    passes = [("h", "h")] if npass == 1 else [("h", "h"), ("h", "l"), ("l", "h")]

    # last lin index of each class (for stop flags)
    last_lin = {t: max(lin for lin in range(S3) if lin % 4 == t) for t in range(4)}

    with tile.TileContext(nc) as tc:
        with (
            tc.tile_pool(name="persist", bufs=1) as persist,
            tc.tile_pool(name="io", bufs=2) as io,
            tc.tile_pool(name="ps", bufs=1, space="PSUM") as pspool,
        ):
            k1t = {
                s: persist.tile([128, NB, DPAD, HPAD, WPAD2], bf16,
                                tag=f"k1t{s}", name=f"k1t{s}")
                for s in names
            }
            for s in names:
                nc.vector.memset(k1t[s][:, :, :, :, :], 0.0)

            # weights: [128=(g,nsub,j), 32=(nsub,q), 343 taps] per piece;
            # double-buffer over m only in 1-pass mode (SBUF budget)
            nwslot = 2 if npass == 1 else 1
            wt = {}
            for s in names:
                for i in range(nwslot):
                    w = persist.tile([128, 32, S3], bf16,
                                     tag=f"wt{s}{i}", name=f"wt{s}{i}")
                    nc.vector.memset(w[:, :, :], 0.0)
                    wt[(s, i)] = w

            fac = persist.tile([128, 1], f32, tag="fac")
            nc.sync.dma_start(out=fac[:, :], in_=factor[:, :])

            # psum: [od-parity][class] -> [128, 448]
            psumb = [
                [
                    pspool.tile([128, SPT], f32, tag=f"pb{par}{t}",
                                name=f"pb{par}{t}")
                    for t in range(4)
                ]
                for par in range(2)
            ]
            # valid-window mode leaves some psum elements unwritten in a
            # round (their true partial is 0 or they are the w-pad junk
            # column); a one-time zero fill keeps those reads defined
            for par in range(2):
                for t in range(4):
                    nc.vector.memset(psumb[par][t][:, :], 0.0)

            for m in range(M_PER_CORE):
                for s in names:
                    for p in range(NB):
                        src_p = k1d[s][m * NB + p, :, :].rearrange(
                            "c (d h w) -> c d h w", d=KS, h=KS, w=KS
                        )
                        for d in range(KS):
                            nc.sync.dma_start(
                                out=k1t[s][:, p, 3 + d, 3:3 + KS, 3:3 + KS],
                                in_=src_p[:, d, :, :],
                            )
                wm = {s: wt[(s, m % nwslot)] for s in names}
                for s in names:
                    for n in range(16):
                        nc.sync.dma_start(
                            out=wm[s][n * NB:(n + 1) * NB,
                                      (n % 4) * NB:(n % 4 + 1) * NB, :],
                            in_=k2d[s][
                                m * NB:(m + 1) * NB, n * NB:(n + 1) * NB, :
                            ].rearrange("q j t -> j q t"),
                        )

                sh = io.tile([128, NB, SP], f32, tag="shell")
                nc.sync.dma_start(
                    out=sh[:, :, :],
                    in_=shell[m * NB:(m + 1) * NB, :, :].rearrange("p c s -> c p s"),
                )
                shf = io.tile([128, NB, SP], f32, tag="shellf")
                nc.vector.tensor_scalar_mul(shf[:, :, :], sh[:, :, :], fac[:, 0:1])

                ost = io.tile([128, NB, KS, KS, WPAD], f32, tag="ost")

                for od in range(KS):
                    par = od % 2
                    # valid windows: contributions are zero unless the
                    # padded read index lands in the 7^3 interior [3,10)
                    kds = [kd for kd in range(KS) if 3 <= od + kd <= 9]
                    # psum bank free layout is (oh, p, ow8) so an
                    # oh-window slice stays a contiguous slab (the sim's
                    # matmul needs 2D-flattenable psum dst APs).  Each
                    # class t starts with a full-oh tap (kh=3; class of
                    # (kd,3,kw) is (kd+1+kw)%4) so the accumulation
                    # group's first matmul covers the whole bank.
                    firsts = []
                    for t in range(4):
                        kd0 = kds[0]
                        kw0 = (t - kd0 - 1) % 4
                        firsts.append(kd0 * KS * KS + 3 * KS + kw0)
                    assert sorted(l % 4 for l in firsts) == [0, 1, 2, 3]
                    ordered = firsts + [
                        lin
                        for kd in kds
                        for lin in range(kd * KS * KS, (kd + 1) * KS * KS)
                        if lin not in set(firsts)
                    ]
                    last_lin_od = {t: max(l for l in ordered if l % 4 == t)
                                   for t in range(4)}
                    for i, lin in enumerate(ordered):
                        kd, r = divmod(lin, KS * KS)
                        kh, kw = divmod(r, KS)
                        oh0, oh1 = max(0, 3 - kh), min(KS, 10 - kh)
                        t = lin % 4
                        first = i < 4
                        last = lin == last_lin_od[t]
                        for g in range(4):
                            c = (g + t) % 4
                            dst = psumb[par][t][
                                32 * c:32 * c + 32, :
                            ].rearrange(
                                "c (oh p ow) -> c oh p ow", oh=KS, p=NB,
                            )[:, oh0:oh1, :, :]
                            for ip, (ws, ks) in enumerate(passes):
                                rhs = k1t[ks][
                                    32 * g:32 * g + 32, :,
                                    od + kd,
                                    kh + oh0:kh + oh1,
                                    kw:kw + WPAD,
                                ].transpose([0, 2, 1, 3])  # (oh, p, ow)
                                nc.tensor.matmul(
                                    dst,
                                    wm[ws][32 * g:32 * g + 32, :, lin],
                                    rhs,
                                    start=first and ip == 0,
                                    stop=last and ip == npass - 1,
                                    tile_position=(32 * g, 32 * c),
                                    # sim group-check is per 2KB
                                    # zero-region; per-strip groups are
                                    # safe on HW (num_active_cols=32)
                                    skip_group_check=True,
                                )
                    # combine rotated partials into ost[:, :, od, :, :].
                    # engines cannot partition-shift, so: aligned DVE
                    # evacuation psum->SBUF, partition-rotating
                    # SBUF->SBUF DMAs, then aligned adds.
                    ev = [
                        io.tile([128, SPT], f32, tag=f"ev{t}", name=f"ev{t}")
                        for t in range(4)
                    ]
                    for t in range(4):
                        nc.vector.tensor_copy(ev[t][:, :], psumb[par][t][:, :])
                    rt = [ev[0]]
                    for t in range(1, 4):
                        r = io.tile([128, SPT], f32, tag=f"rt{t}", name=f"rt{t}")
                        sh4 = 32 * t
                        nc.sync.dma_start(
                            out=r[0:128 - sh4, :], in_=ev[t][sh4:128, :]
                        )
                        nc.sync.dma_start(
                            out=r[128 - sh4:128, :], in_=ev[t][0:sh4, :]
                        )
                        rt.append(r)
                    o_sl = ost[:, :, od, :, :]
                    srcs = [
                        r.rearrange("c (oh p ow) -> c p oh ow", oh=KS, p=NB)
                        for r in rt
                    ]
                    nc.vector.tensor_add(o_sl, srcs[0], srcs[1])
                    nc.vector.tensor_add(o_sl, o_sl, srcs[2])
                    nc.vector.tensor_add(o_sl, o_sl, srcs[3])

                ostf = ost.rearrange("c p a b w -> c p (a b w)")
                nc.vector.tensor_mul(ostf[:, :, :], ostf[:, :, :], shf[:, :, :])
                nc.sync.dma_start(
                    out=out[m * NB:(m + 1) * NB, :, :].rearrange("p c s -> c p s"),
                    in_=ostf[:, :, :],
                )
    nc.compile()
    return nc


def _get_nc(mode=None):
    if mode is None:
        mode = MODE
    if mode not in _CACHE:
        if mode in ("bf16t16", "bf16x3t16", "fp16t16"):
            _CACHE[mode] = _build_nc_t16(mode)
        else:
            _CACHE[mode] = _build_nc(mode)
    return _CACHE[mode]


def _make_in_maps(k1, k2, shell, factor, mode=None):
    import ml_dtypes

    if mode is None:
        mode = MODE

    k1 = np.ascontiguousarray(k1.reshape(128, 128, S3), np.float32)
    k2 = np.ascontiguousarray(k2.reshape(128, 128, S3), np.float32)
    shell_p = np.zeros((128, 128, KS, KS, WPAD), np.float32)
    shell_p[..., :KS] = shell.reshape(128, 128, KS, KS, KS)
    shell_p = shell_p.reshape(128, 128, SP)
    fac = np.full((128, 1), np.float32(factor.reshape(-1)[0]), np.float32)
    rows = M_PER_CORE * NB

    common = {"shell": shell_p, "factor": fac}
    if mode in ("f32r", "f32"):
        k1_pad = np.zeros((128, 128, DPAD, HPAD, WPAD2), np.float32)
        k1_pad[:, :, 3:3 + KS, 3:3 + KS, 3:3 + KS] = k1.reshape(
            128, 128, KS, KS, KS
        )
        k1_pad = k1_pad.reshape(128, 128, PADVOL)
        zeros = np.zeros((128, 128 * KS * KS), np.float32)
        per_full = {"k1pad": k1_pad, "k2": k2, **common}
        shared = {"zeros": zeros}
    else:
        bf = np.float16 if mode == "fp16t16" else ml_dtypes.bfloat16
        k1h = k1.astype(bf)
        k2h = k2.astype(bf)
        per_full = {"k1h": k1h, "k2h": k2h, **common}
        if mode == "bf16x3t16":
            per_full["k1l"] = (k1 - k1h.astype(np.float32)).astype(bf)
            per_full["k2l"] = (k2 - k2h.astype(np.float32)).astype(bf)
        shared = {}

    maps = []
    for c in range(N_CORES):
        m = {k: v[c * rows:(c + 1) * rows] for k, v in per_full.items()
             if k != "factor"}
        m["factor"] = fac
        m.update(shared)
        maps.append(m)
    return maps


def _gather(results):
    outs = [np.asarray(r["out"]) for r in results]
    full = np.concatenate(outs, axis=0)          # (128, 128, 392)
    full = full.reshape(128, 128, KS, KS, WPAD)[..., :KS]
    return np.ascontiguousarray(full)


def kernel(k1, k2, shell, factor, _trace=False):
    from concourse.bass_utils import run_bass_kernel_spmd

    nc = _get_nc(MODE)
    in_maps = _make_in_maps(
        np.asarray(k1), np.asarray(k2), np.asarray(shell), np.asarray(factor),
        mode=MODE,
    )
    try:
        res = run_bass_kernel_spmd(
            nc, in_maps, core_ids=list(range(N_CORES)), trace=_trace
        )
    except ModuleNotFoundError:
        # no NTFF profiling hook in this container; run without trace
        res = run_bass_kernel_spmd(
            nc, in_maps, core_ids=list(range(N_CORES)), trace=False
        )
    out = _gather(res.results)
    if _trace:
        return out, res
    return out
